# revision 12
# baseline (speedup 1.0000x reference)
"""ContextualAttention score kernel for 8 Trainium2 NeuronCores.

Math (per batch): score[p, q] = softmax_p( s10[p] * y[p,q] ) * mm[p], where
  y[p,q]  = sum_{c,di,dj} b_pad[c,pi+di,pj+dj] * f_pad[c,qi+di,qj+dj]
  s10[p]  = 10 * mm[p] / sqrt(sum(w_p^2) + 1152e-4)
  mm[p]   = (mask patch sum == 0)

Sharding: core c -> (batch = c//2, q-half = c%2). No collectives (softmax
is over p, which every core holds in full).

Layout: out[q, p] with q on partitions, p on the free dim, both packed
tight (p = 4096 = 8 PSUM banks of 512). The softmax over p is then a pure
free-dim reduction:
 - 16 q-chunks of 128 (2 grid rows); per chunk 8 p-banks x 9 offsets of
   fp16 matmuls (stationary = repacked contiguous f window [128c,128q],
   moving = b window [128c, 8, 64] strided on the 66-padded grid).
 - fp16 operands: ~2e-3 worst output error (validated vs fp32 reference,
   gate is 2e-2); LDWEIGHTS is 1 cycle/row vs ~3.5 for fp32r.
 - tensor_tensor_reduce fuses z = psum * s10_bc with the running max;
   one Exp activation per chunk computes e and the row sum (accum_out);
   one scalar_tensor_tensor applies recip * mask.
 - per-column max subtraction makes the softmax overflow-safe (logits
   reach ~200 here; the old no-shift scheme produced inf/NaN).
"""

import os
import numpy as np

import concourse.bass as bass
import concourse.bacc as bacc
import concourse.mybir as mybir
import concourse.tile as tile
from concourse import bass_utils

F32 = mybir.dt.float32
F32R = mybir.dt.float32r
F16 = mybir.dt.float16
AF = mybir.ActivationFunctionType
ALU = mybir.AluOpType

C = 128
HP = 66                      # padded image width/height
FLAT = HP * HP + 4           # 4360
NP = 4096                    # tight p positions
NB = 8                       # psum banks / p-tiles of 512
NQC = 16                     # q-chunks per core (128 q each = 2 grid rows)
FROWS = 34                   # f rows per core: 32 + 2 halo
FFLAT = FROWS * HP           # 2244
EPS_SUM = 1152e-4
SCALE = 10.0
OFFS = [(di, dj) for di in range(3) for dj in range(3)]

LAST_EXEC_NS = None
LAST_RES = None
_CACHE = {}


def _build():
    if "nc" in _CACHE:
        return _CACHE["nc"]
    nc = bacc.Bacc(trn_type="TRN2", target_bir_lowering=False, debug=False)

    bp_d = nc.dram_tensor("bp", [C, FLAT], F16, kind="ExternalInput").ap()
    fp_d = nc.dram_tensor("fp", [C, FFLAT], F16, kind="ExternalInput").ap()
    mp_d = nc.dram_tensor("mp", [1, FLAT], F32, kind="ExternalInput").ap()
    out_d = nc.dram_tensor("out", [NQC * C, NP], F16, kind="ExternalOutput").ap()

    with tile.TileContext(nc) as tc:
        with (
            tc.tile_pool(name="small", bufs=1) as small,
            tc.tile_pool(name="img", bufs=1) as img,
            tc.tile_pool(name="rows", bufs=1) as rows,
            tc.tile_pool(name="sl", bufs=2) as slp,
            tc.tile_pool(name="stk", bufs=1) as stk,
            tc.tile_pool(name="stq", bufs=18) as stqp,
            tc.tile_pool(name="zp", bufs=2) as zp,
            tc.tile_pool(name="ep", bufs=2) as ep,
            tc.tile_pool(name="op", bufs=2) as op,
            tc.tile_pool(name="cs", bufs=2) as csp,
            tc.tile_pool(name="ps", bufs=1, space="PSUM") as psp,
        ):
            # ---- constants ----
            ones128_h = small.tile([C, 1], F16, name="ones128_h")
            nc.vector.memset(ones128_h[:, :], 1.0)
            ones9_f = small.tile([9, 1], F32, name="ones9_f")
            nc.vector.memset(ones9_f[:, :], 1.0)
            ones1_f = small.tile([1, C], F32, name="ones1_f")
            nc.vector.memset(ones1_f[:, :], 1.0)

            # ---- images ----
            b16 = img.tile([C, FLAT], F16, name="b16")
            nc.gpsimd.dma_start(b16[:, :], bp_d[:, :])
            f16 = img.tile([C, FFLAT], F16, name="f16")
            nc.gpsimd.dma_start(f16[:, :], fp_d[:, :])
            b_v = b16[:, :HP * HP].rearrange("c (h w) -> c h w", h=HP, w=HP)
            f_v = f16.rearrange("c (h w) -> c h w", h=FROWS, w=HP)

            # psum tiles: 8 banks, reused by name
            def ps_tile(i):
                return psp.tile([C, 512], F32, name=f"ps{i}")

            # ---- preamble: s10 row + mm row, then broadcast ----
            # scs[x] = sum_c b[c,x]^2 as a [1, FLAT] row
            sq16 = img.tile([C, FLAT], F16, name="sq16")
            nc.scalar.activation(sq16[:, :], b16[:, :], AF.Square)
            scs_sb = rows.tile([1, FLAT], F32, name="scs_sb")
            off = 0
            while off < FLAT:
                ln = min(512, FLAT - off)
                pst = ps_tile(0)
                nc.tensor.matmul(pst[0:1, :ln], ones128_h[:, :],
                                 sq16[:, off:off + ln], start=True, stop=True)
                nc.scalar.copy(scs_sb[0:1, off:off + ln], pst[0:1, :ln])
                off += ln
            # mask row
            mp_s = rows.tile([1, FLAT], F32, name="mp_s")
            nc.gpsimd.dma_start(mp_s[:, :], mp_d[:, :])

            # 3x3 window stacks [9, NP] (tight p indexing)
            sstk = stk.tile([9, NP], F32, name="sstk")
            mstk = stk.tile([9, NP], F32, name="mstk")
            for di in range(3):
                for dj in range(3):
                    o9 = di * HP + dj
                    src_s = bass.AP(tensor=scs_sb.tensor,
                                    offset=scs_sb.offset + o9,
                                    ap=[[FLAT, 1], [HP, 64], [1, 64]])
                    nc.gpsimd.dma_start(sstk[3 * di + dj:3 * di + dj + 1, :],
                                        src_s)
                    src_m = bass.AP(tensor=mp_s.tensor,
                                    offset=mp_s.offset + o9,
                                    ap=[[FLAT, 1], [HP, 64], [1, 64]])
                    nc.gpsimd.dma_start(mstk[3 * di + dj:3 * di + dj + 1, :],
                                        src_m)

            epsb = small.tile([1, 1], F32, name="epsb")
            nc.vector.memset(epsb[:, :], EPS_SUM)

            # per-512-slice: den2/pm -> s10/mm slice -> broadcast to [128, NP]
            s10_bc = img.tile([C, NP], F32, name="s10_bc")
            mm_bc = img.tile([C, NP], F32, name="mm_bc")
            for t in range(NB):
                sl = 512 * t
                pst_a = ps_tile(5 + (t % 2))
                nc.tensor.matmul(pst_a[0:1, :], ones9_f[:, :],
                                 sstk[:, sl:sl + 512], start=True, stop=True)
                pst_b = ps_tile(t % 2)
                nc.tensor.matmul(pst_b[0:1, :], ones9_f[:, :],
                                 mstk[:, sl:sl + 512], start=True, stop=True)
                den_s = slp.tile([1, 512], F32, name="den_s")
                nc.scalar.activation(den_s[0:1, :], pst_a[0:1, :], AF.Sqrt,
                                     bias=epsb[0:1, :])
                rden_s = slp.tile([1, 512], F32, name="rden_s")
                nc.vector.reciprocal(rden_s[0:1, :], den_s[0:1, :])
                mm_s = slp.tile([1, 512], F32, name="mm_s")
                nc.vector.tensor_scalar(mm_s[0:1, :], pst_b[0:1, :], 0.0,
                                        None, ALU.is_equal)
                s10_s = slp.tile([1, 512], F32, name="s10_s")
                nc.vector.scalar_tensor_tensor(s10_s[0:1, :], rden_s[0:1, :],
                                               SCALE, mm_s[0:1, :],
                                               op0=ALU.mult, op1=ALU.mult)
                pbc = ps_tile(2 + (t % 2))
                nc.tensor.matmul(pbc[:, :], ones1_f[:, :],
                                 s10_s[0:1, :], start=True, stop=True)
                nc.scalar.copy(s10_bc[:, sl:sl + 512], pbc[:, :])
                pbc2 = ps_tile(4 + (t % 2) * 3)
                nc.tensor.matmul(pbc2[:, :], ones1_f[:, :],
                                 mm_s[0:1, :], start=True, stop=True)
                nc.scalar.copy(mm_bc[:, sl:sl + 512], pbc2[:, :])

            # ---- main loop over q-chunks ----
            STAGE = int(os.environ.get("KBENCH_STAGE", "99"))
            for j in range(NQC if STAGE >= 2 else 0):
                # repack 9 contiguous stationaries [128, 128] fp16
                sts = []
                for (di, dj) in OFFS:
                    stq = stqp.tile([C, C], F16, name="stq")
                    stq_v = stq.rearrange("c (a b) -> c a b", a=2, b=64)
                    nc.vector.tensor_copy(stq_v[:, :, :],
                                          f_v[:, 2 * j + di:2 * j + di + 2,
                                              dj:dj + 64])
                    sts.append(stq)

                o_t = op.tile([C, NP], F16, name="o_t")
                z = zp.tile([C, NP], F32, name="z")
                mx = csp.tile([C, NB], F32, name="mx")
                for pt in range(NB):
                    pst = ps_tile(pt)
                    for o, (di, dj) in enumerate(OFFS):
                        nc.tensor.matmul(
                            pst[:, :], sts[o][:, :],
                            b_v[:, 8 * pt + di:8 * pt + di + 8, dj:dj + 64],
                            start=(o == 0), stop=(o == 8))
                    if STAGE == 2:
                        nc.scalar.copy(o_t[:, 512 * pt:512 * pt + 512],
                                       pst[:, :])
                        continue
                    zs = z[:, 512 * pt:512 * pt + 512]
                    nc.vector.scalar_tensor_tensor(
                        zs, pst[:, :], 1.0, s10_bc[:, 512 * pt:512 * pt + 512],
                        op0=ALU.mult, op1=ALU.mult)
                    nc.vector.tensor_reduce(mx[:, pt:pt + 1], zs,
                                            axis=mybir.AxisListType.X,
                                            op=ALU.max)

                if STAGE >= 3:
                    mall = csp.tile([C, 1], F32, name="mall")
                    nc.vector.tensor_reduce(mall[:, :], mx[:, :],
                                            axis=mybir.AxisListType.X,
                                            op=ALU.max)
                    negm = csp.tile([C, 1], F32, name="negm")
                    nc.vector.tensor_scalar(negm[:, :], mall[:, :], -1.0,
                                            None, ALU.mult)
                if STAGE == 3:
                    nc.scalar.copy(o_t[:, :], z[:, :])
                if STAGE >= 4:
                    e = ep.tile([C, NP], F16, name="e")
                    ssum = csp.tile([C, 1], F32, name="ssum")
                    nc.scalar.activation(e[:, :], z[:, :], AF.Exp,
                                         bias=negm[:, :], accum_out=ssum[:, :])
                    recip = csp.tile([C, 1], F32, name="recip")
                    nc.vector.reciprocal(recip[:, :], ssum[:, :])
                if STAGE == 4:
                    nc.scalar.copy(o_t[:, :], e[:, :])
                if STAGE >= 5:
                    nc.vector.scalar_tensor_tensor(o_t[:, :], e[:, :],
                                                   recip[:, :], mm_bc[:, :],
                                                   op0=ALU.mult, op1=ALU.mult)
                nc.gpsimd.dma_start(out_d[C * j:C * j + C, :], o_t[:, :])

    nc.compile()
    _CACHE["nc"] = nc
    return nc


def _prep_inputs(f, b, mask):
    f = np.asarray(f, np.float32)
    b = np.asarray(b, np.float32)
    mask = np.asarray(mask, np.float32)

    mask_s = mask[0, 0, ::8, ::8]                       # batch 0, as in source
    mp = np.zeros((1, FLAT), np.float32)
    mpv = mp[0, :HP * HP].reshape(HP, HP)
    mpv[1:65, 1:65] = mask_s

    in_maps = []
    for c in range(8):
        bi, h = c // 2, c % 2
        bpad = np.zeros((C, FLAT), np.float16)
        bpv = bpad[:, :HP * HP].reshape(C, HP, HP)
        bpv[:, 1:65, 1:65] = b[bi]
        fpad = np.zeros((C, HP, HP), np.float16)
        fpad[:, 1:65, 1:65] = f[bi]
        fcore = np.ascontiguousarray(
            fpad[:, 32 * h:32 * h + FROWS, :].reshape(C, FFLAT))
        in_maps.append({"bp": bpad, "fp": fcore, "mp": mp})
    return in_maps


def kernel(f, b, mask):
    global LAST_EXEC_NS
    nc = _build()
    in_maps = _prep_inputs(f, b, mask)
    trace = bool(int(os.environ.get("KBENCH_TRACE", "0")))
    res = bass_utils.run_bass_kernel_spmd(
        nc, in_maps, core_ids=list(range(8)), trace=trace)
    LAST_EXEC_NS = res.exec_time_ns
    globals()["LAST_RES"] = res

    B = np.asarray(f).shape[0]
    out = np.empty((B, NP, 4096), np.float32)
    for c in range(8):
        bi, h = c // 2, c % 2
        oc = np.asarray(res.results[c]["out"], np.float32)   # [2048 q, 4096 p]
        out[bi, :, 2048 * h:2048 * (h + 1)] = oc.T
    return out.reshape(B, NP, 64, 64)
